# revision 22
# baseline (speedup 1.0000x reference)
"""Multi-head self-attention (b=4, n=2048, d=512, h=8, dh=64) on 8 trn2 cores.

Sharding: core c -> (batch b = c//2, sequence half s = c%2). Each core computes
the full K/V for its batch (recomputed on both cores of the pair -- cheaper
than exchanging) and queries for its own half of the sequence, so outputs are
disjoint row blocks and the host-side gather is a pure reshape (no reduction).

Per-core kernel (all matmuls bf16 with f32 PSUM accumulation):
  xT [512, 2048]  (x[b].T with the core's own q-half in columns 0..1023; kv
                   column order is permuted per-core, which softmax attention
                   is invariant to)
  QT = Wq^T @ xT[:, :1024]          [512, 1024]   (heads on partition rows)
  KT = Wk^T @ xT                    [512, 2048]
  V  = x[b] @ Wv                    [2048, 512]   (kv rows on partitions)
  per head h: S^T = K_h Q_h^T       [2048, 1024]  scores, kv on partitions
              E = exp(S^T / 8)      (no max subtraction; scores are O(1))
              O_aug^T = [V_h | 1]^T @ E   [65, 1024]  row 64 = softmax denors
              O^T_h = O^T / denom   (reciprocal + partition broadcast)
  out = O^T_all^T @ Wo + bo         [1024, 512]
"""

import sys

sys.path.insert(0, "/opt/trn_rl_repo")

from contextlib import ExitStack

import ml_dtypes
import numpy as np

import concourse.bass as bass
import concourse.tile as tile
from concourse import bacc, mybir
from concourse.bass import ts, ds
from concourse.bass_utils import run_bass_kernel_spmd

BF16 = mybir.dt.bfloat16
F32 = mybir.dt.float32

D = 512        # model dim
H = 8          # heads
DH = 64        # head dim
NQ = 1024      # q rows per core
NKV = 2048     # kv rows per core
P = 128
SCALE = DH ** -0.5


def build_nc(finalize=True, dbg=False):
    nc = bacc.Bacc("TRN2", target_bir_lowering=False)

    # inputs arrive pre-arranged as [partition, chunk, free] so the DMAs are
    # fully contiguous per partition
    xT_d = nc.dram_tensor("xT", [P, D // P, NKV], BF16, kind="ExternalInput")
    Wq_d = nc.dram_tensor("Wq", [P, D // P, D], BF16, kind="ExternalInput")
    Wk_d = nc.dram_tensor("Wk", [P, D // P, D], BF16, kind="ExternalInput")
    Wv_d = nc.dram_tensor("Wv", [P, D // P, D], BF16, kind="ExternalInput")
    Wo_d = nc.dram_tensor("Wo", [P, D // P, D], BF16, kind="ExternalInput")
    bo_d = nc.dram_tensor("bo", [1, D], F32, kind="ExternalInput")
    out_d = nc.dram_tensor("out", [NQ, D], F32, kind="ExternalOutput")
    if dbg:
        QT_o = nc.dram_tensor("QT_o", [P, D // P, NQ], BF16, kind="ExternalOutput")
        KT_o = nc.dram_tensor("KT_o", [P, D // P, NKV], BF16, kind="ExternalOutput")
        Va_o = nc.dram_tensor("Va_o", [P, NKV // P, H, DH + 1], BF16, kind="ExternalOutput")
        OT_o = nc.dram_tensor("OT_o", [P, D // P, NQ], BF16, kind="ExternalOutput")

    KO = D // P  # 4 outer chunks of the model dim

    with tile.TileContext(nc) as tc, ExitStack() as ctx:
        consts = ctx.enter_context(tc.tile_pool(name="consts", bufs=1))
        ps = ctx.enter_context(tc.tile_pool(name="ps", bufs=2, space="PSUM"))
        avps = ctx.enter_context(tc.tile_pool(name="avps", bufs=2, space="PSUM"))
        expp = ctx.enter_context(tc.tile_pool(name="expp", bufs=18))
        small = ctx.enter_context(tc.tile_pool(name="small", bufs=2))
        outp = ctx.enter_context(tc.tile_pool(name="outp", bufs=2))

        # ---- persistent SBUF tensors ----
        xT_sb = consts.tile([P, KO, NKV], BF16, tag="xT")
        Wq_sb = consts.tile([P, KO, D], BF16, tag="Wq")
        Wk_sb = consts.tile([P, KO, D], BF16, tag="Wk")
        Wv_sb = consts.tile([P, KO, D], BF16, tag="Wv")
        Wo_sb = consts.tile([P, KO, D], BF16, tag="Wo")
        bo_sb = consts.tile([1, D], F32, tag="bo")
        bo_bc = consts.tile([P, D], F32, tag="bo_bc")
        QT_sb = consts.tile([P, KO, NQ], BF16, tag="QT")
        KT_sb = consts.tile([P, KO, NKV], BF16, tag="KT")
        # V with a ones column per head: [kv_part, kv_outer, head, dh+1]
        Vaug_sb = consts.tile([P, NKV // P, H, DH + 1], BF16, tag="Vaug")
        OT_sb = consts.tile([P, KO, NQ], BF16, tag="OT")

        # spread the input DMAs over several queues so they land in parallel
        nc.sync.dma_start(bo_sb[:], bo_d[:])
        nc.sync.dma_start(xT_sb[:, :, 0:NQ], xT_d[:, :, 0:NQ])
        nc.sync.dma_start(Wq_sb[:], Wq_d[:])
        nc.sync.dma_start(Wk_sb[:], Wk_d[:])
        nc.sync.dma_start(xT_sb[:, :, NQ:NKV], xT_d[:, :, NQ:NKV])
        nc.sync.dma_start(Wv_sb[:], Wv_d[:])
        nc.sync.dma_start(Wo_sb[:], Wo_d[:])
        nc.gpsimd.partition_broadcast(bo_bc[:], bo_sb[:])
        # spin the PE for ~3.5us on junk so HAM unthrottles before the
        # first real matmuls
        junk = small.tile([64, 64], BF16, tag="junk")
        nc.vector.memset(junk[:], 0.0)
        wp = ps.tile([P, 512], F32, tag="ps", name="wp")
        for _ in range(84):
            nc.tensor.matmul(wp[0:64, 0:64], lhsT=junk[:], rhs=junk[:],
                             start=True, stop=True)
        nc.vector.memset(Vaug_sb[:, :, :, DH : DH + 1], 1.0)
        # touch the exp table early so the ~2.7us ACT_TABLE_LOAD overlaps DMAs
        warm = small.tile([1, 8], F32, tag="warm")
        nc.scalar.activation(warm[:], bo_sb[0:1, 0:8],
                             mybir.ActivationFunctionType.Exp)

        # ---- projections: Q chunk 0 and K chunk 0 first so attention can start ----
        def q_proj(o):
            qp = avps.tile([P, NQ], F32, tag="avps")
            for n in range(NQ // 512):
                for k in range(KO):
                    nc.tensor.matmul(
                        qp[:, ts(n, 512)],
                        lhsT=Wq_sb[:, k, ts(o, P)],
                        rhs=xT_sb[:, k, ts(n, 512)],
                        start=(k == 0),
                        stop=(k == KO - 1),
                    )
            nc.vector.tensor_copy(QT_sb[:, o, :], qp[:])

        def k_proj(o, n2):
            kp = avps.tile([P, 1024], F32, tag="avps")
            for n in range(2):
                for k in range(KO):
                    nc.tensor.matmul(
                        kp[:, ts(n, 512)],
                        lhsT=Wk_sb[:, k, ts(o, P)],
                        rhs=xT_sb[:, k, ds(n2 * 1024 + n * 512, 512)],
                        start=(k == 0),
                        stop=(k == KO - 1),
                    )
            nc.vector.tensor_copy(KT_sb[:, o, ds(n2 * 1024, 1024)], kp[:])

        def v_proj(j):
            vp = avps.tile([P, 512], F32, tag="avps")
            for k in range(KO):
                nc.tensor.matmul(
                    vp[:],
                    lhsT=xT_sb[:, k, ts(j, P)],
                    rhs=Wv_sb[:, k, :],
                    start=(k == 0),
                    stop=(k == KO - 1),
                )
            nc.vector.tensor_copy(
                Vaug_sb[:, j, :, 0:DH], vp.rearrange("p (h d) -> p h d", h=H)
            )

        q_proj(0)
        k_proj(0, 0)
        k_proj(0, 1)

        JT = NKV // P  # 16 kv tiles per head
        JH = 2         # exp tiles hold 2 kv tiles for fine-grained release

        avp_tiles = {}
        exp_tiles = {}

        def av_quarter(h, k):
            """Four kv tiles (2 exp tiles) of O_aug^T = [V_h | 1]^T @ E."""
            if k == 0:
                avp_tiles[h] = avps.tile([P, NQ], F32, tag="avps", name=f"avp{h}")
            avp = avp_tiles[h]
            for j in range(4 * k, 4 * k + 4):
                et = exp_tiles[h][j // JH]
                for n in range(NQ // 512):
                    nc.tensor.matmul(
                        avp[0 : DH + 1, ts(n, 512)],
                        lhsT=Vaug_sb[:, j, h, :],
                        rhs=et[:, j % JH, ts(n, 512)],
                        start=(j == 0),
                        stop=(j == JT - 1),
                    )

        def av_finish(h):
            """Normalize by the softmax denominator (row 64) and place O^T_h.

            partition_broadcast only works from partition 0 on hardware, and
            DVE handles 32-aligned cross-partition-base operands, so the
            chain runs on DVE/GpSimd with no DMA bounces.
            """
            i, half = h // 2, h % 2
            avp = avp_tiles.pop(h)
            del exp_tiles[h]
            rs0 = small.tile([1, NQ], F32, tag="rs0")
            nc.vector.tensor_copy(rs0[:], avp[DH : DH + 1, :])
            rc0 = small.tile([1, NQ], F32, tag="rc0")
            nc.vector.reciprocal_approx_fast(rc0[:], rs0[:])
            rb = small.tile([DH, NQ], F32, tag="rb")
            nc.gpsimd.partition_broadcast(rb[:], rc0[:])
            nc.vector.tensor_mul(
                OT_sb[64 * half : 64 * half + DH, i, :], avp[0:DH, :], rb[:]
            )

        NPAIR = H // 2

        # Per-(pair, j) extra PE work interleaved into the exp-paced stream:
        # pair 0 carries the V projection + chunk-1 Q/K projections; pairs
        # 1..2 carry the previous pair's AV + later Q/K chunks; pair 3
        # drains its own heads' AV with a one-quarter lag.
        extras = {}

        def addx(i, j, fn):
            extras.setdefault((i, j), []).append(fn)

        for j in range(JT):
            addx(0, j, lambda j=j: v_proj(j))
        addx(0, 5, lambda: q_proj(1))
        addx(0, 9, lambda: k_proj(1, 0))
        addx(0, 13, lambda: k_proj(1, 1))
        addx(1, 10, lambda: q_proj(2))
        addx(1, 12, lambda: k_proj(2, 0))
        addx(1, 14, lambda: k_proj(2, 1))
        addx(2, 10, lambda: q_proj(3))
        addx(2, 12, lambda: k_proj(3, 0))
        addx(2, 14, lambda: k_proj(3, 1))
        for i in (1, 2, 3):
            ha, hb = 2 * (i - 1), 2 * (i - 1) + 1
            for k in range(4):
                addx(i, 1 + 2 * k, lambda h=ha, k=k: av_quarter(h, k))
                addx(i, 2 + 2 * k, lambda h=hb, k=k: av_quarter(h, k))
            addx(i, 8, lambda h=ha: av_finish(h))
            addx(i, 9, lambda h=hb: av_finish(h))
        # pair 3 additionally drains its own heads early (quarters k need
        # exps through kv tile 4k+3, ready at j=4k+3)
        for k in range(3):
            addx(3, 9 + 2 * k, lambda k=k: av_quarter(6, k))
            addx(3, 10 + 2 * k, lambda k=k: av_quarter(7, k))

        for i in range(NPAIR):
            # scores + exp for both heads of the pair, interleaved so the
            # K=64 matmuls run concurrently in different PE row groups
            ia, ib = 2 * i, 2 * i + 1
            pa, pb = slice(0, 64), slice(64, 128)
            for j in range(JT):
                if j % JH == 0:
                    if j == 0:
                        exp_tiles[ia] = []
                        exp_tiles[ib] = []
                    exp_tiles[ia].append(expp.tile(
                        [P, JH, NQ], BF16, tag="expS", name=f"expA{i}_{j // JH}"))
                    exp_tiles[ib].append(expp.tile(
                        [P, JH, NQ], BF16, tag="expS", name=f"expB{i}_{j // JH}"))
                spa = ps.tile([P, NQ], F32, tag="ps")
                spb = ps.tile([P, NQ], F32, tag="ps")
                for n in range(NQ // 512):
                    nc.tensor.matmul(
                        spa[:, ts(n, 512)], lhsT=KT_sb[pa, i, ts(j, P)],
                        rhs=QT_sb[pa, i, ts(n, 512)], start=True, stop=True,
                    )
                    nc.tensor.matmul(
                        spb[:, ts(n, 512)], lhsT=KT_sb[pb, i, ts(j, P)],
                        rhs=QT_sb[pb, i, ts(n, 512)], start=True, stop=True,
                    )
                nc.scalar.activation(
                    exp_tiles[ia][j // JH][:, j % JH], spa[:],
                    mybir.ActivationFunctionType.Exp, scale=SCALE,
                )
                nc.scalar.activation(
                    exp_tiles[ib][j // JH][:, j % JH], spb[:],
                    mybir.ActivationFunctionType.Exp, scale=SCALE,
                )
                for fn in extras.get((i, j), []):
                    fn()

        # tail: last exp-gated AV quarters + normalize
        av_quarter(6, 3)
        av_quarter(7, 3)
        av_finish(6)
        av_finish(7)

        if dbg:
            nc.sync.dma_start(QT_o[:], QT_sb[:])
            nc.sync.dma_start(KT_o[:], KT_sb[:])
            nc.sync.dma_start(Va_o[:], Vaug_sb[:])
            nc.sync.dma_start(OT_o[:], OT_sb[:])

        # ---- output projection ----
        out_r = out_d.rearrange("(t p) d -> p t d", p=P)
        for qt in range(NQ // P):
            op = ps.tile([P, D], F32, tag="ps")
            for k in range(KO):
                nc.tensor.matmul(
                    op[:],
                    lhsT=OT_sb[:, k, ts(qt, P)],
                    rhs=Wo_sb[:, k, :],
                    start=(k == 0),
                    stop=(k == KO - 1),
                )
            ot = outp.tile([P, D], F32, tag="out")
            nc.vector.tensor_add(ot[:], op[:], bo_bc[:])
            nc.sync.dma_start(out_r[:, qt, :], ot[:])

    if finalize:
        nc.finalize()
    return nc


_NC_CACHE = None


def _get_nc():
    global _NC_CACHE
    if _NC_CACHE is None:
        _NC_CACHE = build_nc()
    return _NC_CACHE


def _chunked(w):
    """[512, N] -> [128, 4, N] with row r at [r % 128, r // 128]."""
    n = w.shape[1]
    return np.ascontiguousarray(w.reshape(D // P, P, n).transpose(1, 0, 2))


def make_in_maps(x, Wq, Wkv, Wo, bo):
    bf = ml_dtypes.bfloat16
    Wq_b = _chunked(np.asarray(Wq, np.float32)).astype(bf)
    Wk_b = _chunked(np.asarray(Wkv[:, :D], np.float32)).astype(bf)
    Wv_b = _chunked(np.asarray(Wkv[:, D:], np.float32)).astype(bf)
    Wo_b = _chunked(np.asarray(Wo, np.float32)).astype(bf)
    bo_f = np.asarray(bo, np.float32).reshape(1, D)
    in_maps = []
    for c in range(8):
        b, s = divmod(c, 2)
        xb = np.asarray(x[b], np.float32)
        own = xb[s * NQ : (s + 1) * NQ]
        other = xb[(1 - s) * NQ : (2 - s) * NQ]
        xT = _chunked(np.concatenate([own, other], axis=0).T).astype(bf)
        in_maps.append(
            {"xT": xT, "Wq": Wq_b, "Wk": Wk_b, "Wv": Wv_b, "Wo": Wo_b, "bo": bo_f}
        )
    return in_maps


def gather_out(results, x):
    b_total, n, _ = x.shape
    out = np.empty((b_total, n, D), np.float32)
    for c in range(8):
        b, s = divmod(c, 2)
        out[b, s * NQ : (s + 1) * NQ] = results[c]["out"]
    return out


def kernel(x, Wq, Wkv, Wo, bo, trace=False):
    nc = _get_nc()
    in_maps = make_in_maps(x, Wq, Wkv, Wo, bo)
    res = run_bass_kernel_spmd(nc, in_maps, core_ids=list(range(8)), trace=trace)
    out = gather_out(res.results, np.asarray(x))
    if trace:
        kernel.last_exec_time_ns = res.exec_time_ns
    return out


kernel.last_exec_time_ns = None
